# revision 2
# baseline (speedup 1.0000x reference)
"""DenseKAN forward as a single fused matmul on TRN2.

Math: the reference uses a uniform knot grid (spacing h=0.4 on
[-2.2, 2.2]), so the Cox-de Boor bases are shifted copies of the
cardinal cubic B-spline with u = 2.5x + 5.5 in [3, 8):

    B_j(x) = Q(u - j),   Q(s) = (1/6) sum_m (-1)^m C(4,m) relu(s-m)^3

Using Q's symmetry Q(s) = Q(4-s), each basis is expanded from the side
that keeps the truncated-power features small (bounded by ~26 after the
1/2.5 rescale, which keeps the binomial cancellation mild enough for
the PE's reduced-precision fp32r mode):

    blocks 0..3:  f_n = max((n-1.5)/2.5 - x, 0)^3   (right-side powers)
    blocks 4..7:  f_n = max(x + (5.5-n)/2.5, 0)^3   (left-side powers)
    block  8:     silu(x)

    B_0 = 2.5^3/6 * f_0            B_7 = 2.5^3/6 * f_7
    B_1 = 2.5^3/6 * (f_1 - 4 f_0)  B_6 = 2.5^3/6 * (f_6 - 4 f_7)  etc.

All coefficients, the per-dim scale factor, and the bias (via partition
of unity, sum_j B_j == 1) are folded into the weights on the host, so
the whole layer is out = F(x) @ W2 with F computed on-chip:
per block one GpSimd dual-op (add,max), one ACT Square, one DVE mul.
The host also pre-transposes x (shipping [x^T | -x^T]) so no on-chip
transpose is needed. Batch is sharded across the 8 cores (128 rows
each); weights are replicated.
"""

import numpy as np

import concourse.bass as bass
import concourse.mybir as mybir
import concourse.tile as tile
from concourse import bacc
from concourse.bass_utils import run_bass_kernel_spmd

BATCH = 1024
IN = 256
UNITS = 256
GK = 8  # number of spline bases per input dim
NF = GK + 1  # + silu feature block
K = IN * NF  # 2304 contraction rows
N_CORES = 8
BS = BATCH // N_CORES  # 128 batch rows per core
KT = K // 128  # 18 K-tiles
W_CHUNKS = (2, 4, 6, 6)
N_WARM = 6  # PE warm-up matmuls (HAM clock-gate burn-in)

FP32 = mybir.dt.float32
MM_DT = mybir.dt.float32r  # matmul compute dtype (fp32 bit layout)

AluOp = mybir.AluOpType

_cache = {}


def _build():
    nc = bacc.Bacc("TRN2", target_bir_lowering=False, debug=False,
                   enable_asserts=False, num_devices=N_CORES)
    # host ships [x^T | -x^T] as the SBUF image: (128, 4*BS)
    xt_d = nc.dram_tensor("xt", [128, 4 * BS], FP32,
                          kind="ExternalInput").ap()
    # host pre-swizzled: w2[p, k, o] = W2_flat[128*k + p, o]
    w_d = nc.dram_tensor("w2", [128, KT, UNITS], MM_DT,
                         kind="ExternalInput").ap()
    o_d = nc.dram_tensor("out", [BS, UNITS], FP32, kind="ExternalOutput").ap()

    with tile.TileContext(nc) as tc:
        with (
            tc.tile_pool(name="const", bufs=1) as cpool,
            tc.tile_pool(name="blk", bufs=3) as bpool,
            tc.tile_pool(name="psum", bufs=1, space="PSUM") as ppool,
        ):
            # x first: the whole feature pipeline hangs off it
            xt = cpool.tile([128, 4 * BS], FP32)
            nc.sync.dma_start(xt[:], xt_d[:])

            # weights stream behind x; first chunk small so the PE can
            # start on the silu block early
            w2 = cpool.tile([128, KT, UNITS], MM_DT)
            lo = 0
            for sz in W_CHUNKS:
                nc.sync.dma_start(w2[:, lo:lo + sz, :], w_d[:, lo:lo + sz, :])
                lo += sz

            # PE warm-up: HAM keeps the PE at 1.2 GHz until ~3.4us of
            # sustained activity; burn that in while the weights stream
            wtile = cpool.tile([128, 512], MM_DT)
            nc.vector.tensor_copy(
                wtile[:], nc.const_aps.tensor(1.0, (128, 512), FP32))
            wpsum = ppool.tile([128, 512], FP32)
            for _ in range(N_WARM):
                nc.tensor.matmul(wpsum[:], wtile[:, 0:128], wtile[:],
                                 start=True, stop=True)

            T = cpool.tile([128, NF * 256], MM_DT)
            opsum = ppool.tile([BS, UNITS], FP32)

            # weight k-tile order (host side matches): silu pair first,
            # then feature blocks in compute order
            nc.scalar.activation(T[:, GK * 256:(GK + 1) * 256],
                                 xt[:, 0:2 * BS],
                                 mybir.ActivationFunctionType.Silu)
            nc.tensor.matmul(opsum[:], T[:, 2048:2176], w2[:, 0, :],
                             start=True, stop=False)
            nc.tensor.matmul(opsum[:], T[:, 2176:2304], w2[:, 1, :],
                             start=False, stop=False)

            for n in range(GK):
                if n < 4:
                    src = xt[:, 2 * BS:4 * BS]  # -x^T
                    c = (n - 1.5) / 2.5
                else:
                    src = xt[:, 0:2 * BS]  # x^T
                    c = (5.5 - n) / 2.5
                t1 = bpool.tile([128, 256], FP32, tag="t1")
                nc.gpsimd.tensor_scalar(t1[:], src, float(c), 0.0,
                                        AluOp.add, AluOp.max)
                sq = bpool.tile([128, 256], FP32, tag="sq")
                nc.scalar.square(sq[:], t1[:])
                blk = T[:, n * 256:(n + 1) * 256]
                nc.vector.tensor_mul(blk, sq[:], t1[:])
                for h in range(2):
                    k = 2 * n + h
                    nc.tensor.matmul(opsum[:],
                                     T[:, k * 128:(k + 1) * 128],
                                     w2[:, 2 + k, :],
                                     start=False, stop=(k == 2 * GK - 1))

            osb = cpool.tile([BS, UNITS], FP32)
            nc.vector.tensor_copy(osb[:], opsum[:])
            nc.sync.dma_start(o_d[:], osb[:])

    nc.compile()
    return nc


def _fold_weights(spline_kernel, scale_factor, bias):
    """-> (128, KT, UNITS) swizzled folded weights, w2[p,k,o]=W2[128k+p,o]."""
    sk = spline_kernel.astype(np.float64)
    sf = scale_factor.astype(np.float64)
    b = bias.astype(np.float64)
    # W[i,j,o] = sk*sf + bias/IN  (bias folded via sum_j B_j == 1)
    W = sk * sf[:, None, :] + b[None, None, :] / IN
    comb = 2.5 ** 3 * np.array([1.0, -4.0, 6.0, -4.0, 1.0]) / 6.0
    # A[j, n] = coefficient of feature-block n in basis j
    A = np.zeros((GK, GK))
    for j in range(4):  # right-side: B_j = sum_m comb[m] * f_{j-m}
        for m in range(j + 1):
            A[j, j - m] = comb[m]
    for j in range(4, GK):  # left-side: B_j = sum_m comb[m] * f_{j+m}
        for m in range(GK - j):
            A[j, j + m] = comb[m]
    W2 = np.einsum("jn,ijo->nio", A, W)  # (GK, IN, UNITS)
    Wfull = np.concatenate([sf[None, :, :], W2], axis=0)  # silu block first
    flat = Wfull.reshape(K, UNITS)
    sw = flat.reshape(KT, 128, UNITS).transpose(1, 0, 2)  # -> [p, k, o]
    return np.ascontiguousarray(sw.astype(np.float32))


def _prep_x(x):
    """(BATCH, IN) -> per-core (128, 4*BS) SBUF images [x^T | -x^T]."""
    x = np.asarray(x, dtype=np.float32)
    outs = []
    for c in range(N_CORES):
        xs = x[c * BS:(c + 1) * BS]  # (BS, IN)
        xtc = np.ascontiguousarray(xs.T)  # (IN, BS)
        b0, b1 = xtc[:128], xtc[128:]
        outs.append(np.ascontiguousarray(
            np.concatenate([b0, b1, -b0, -b1], axis=1)))  # (128, 4*BS)
    return outs


def make_in_maps(inputs):
    w2 = _fold_weights(inputs["spline_kernel"], inputs["scale_factor"],
                       inputs["bias"])
    xts = _prep_x(inputs["x"])
    return [{"xt": xts[c], "w2": w2} for c in range(N_CORES)]


def kernel(x, spline_kernel, scale_factor, bias):
    if "nc" not in _cache:
        _cache["nc"] = _build()
    nc = _cache["nc"]

    in_maps = make_in_maps({"x": x, "spline_kernel": spline_kernel,
                            "scale_factor": scale_factor, "bias": bias})
    res = run_bass_kernel_spmd(nc, in_maps, list(range(N_CORES)))
    out = np.concatenate([res.results[c]["out"] for c in range(N_CORES)],
                         axis=0)
    return out.astype(np.float32)



# revision 4
# speedup vs baseline: 2.1515x; 2.1515x over previous
"""DenseKAN forward as a single fused fp16 matmul on TRN2.

Math: the reference uses a uniform knot grid (spacing h=0.4 on
[-2.2, 2.2]), so the Cox-de Boor bases are shifted copies of the
cardinal cubic B-spline; each basis B_j expands over truncated-power
features f_n = relu(u_n)^3 with

    u_n = (n-1.5)/2.5 - x   (n < 4,  right-side powers)
    u_n = x + (5.5-n)/2.5   (n >= 4, left-side powers)

plus a silu(x) block; all basis coefficients, the per-dim scale factor
and the bias (partition of unity, sum_j B_j == 1) fold into the weights
on the host, so the layer is out = [silu(x) | relu(u)^3 blocks] @ W2.

v2 pipeline (v1 bottleneck was 8 serialized GpSimd tensor_scalar ops at
~3.8us each, which also degraded concurrent DVE ops ~5x): the host
ships the 8 shifted blocks U = [u_0..u_7] as ONE [128, 2048] fp16 image
per core, so on-chip feature work is three WIDE ops per 4-block chunk —
DVE relu (tensor_scalar max), ACT square, DVE multiply — with no GpSimd
involvement. silu(x) is recovered from u_0 via ACT Silu(-u_0 - 0.6).
Everything (U, features, weights) is fp16: rel err ~2.6e-3 (vs 2e-2
budget), weight DMA halves vs fp32, and the PE runs at bf16 rate with
fast weight load. Batch is sharded across the 8 cores (128 rows each);
weights are replicated.
"""

import numpy as np

import concourse.bass as bass
import concourse.mybir as mybir
import concourse.tile as tile
from concourse import bacc
from concourse.bass_utils import run_bass_kernel_spmd

BATCH = 1024
IN = 256
UNITS = 256
GK = 8  # number of spline bases per input dim
NF = GK + 1  # + silu feature block
K = IN * NF  # 2304 contraction rows
N_CORES = 8
BS = BATCH // N_CORES  # 128 batch rows per core
KT = K // 128  # 18 k-tiles
N_WARM = 5  # PE warm-up matmuls (HAM clock-gate burn-in)

FP32 = mybir.dt.float32
F16 = mybir.dt.float16

AluOp = mybir.AluOpType
Act = mybir.ActivationFunctionType

_cache = {}


def _build():
    nc = bacc.Bacc("TRN2", target_bir_lowering=False, debug=False,
                   enable_asserts=False, num_devices=N_CORES)
    # host ships the 8 shifted blocks [u_0 | ... | u_7], each [128, 256]
    xu_d = nc.dram_tensor("xu", [128, GK * 256], F16,
                          kind="ExternalInput").ap()
    # host pre-swizzled: w2[p, k, o] = W2_flat[128*k + p, o], fp16
    w_d = nc.dram_tensor("w2", [128, KT, UNITS], F16,
                         kind="ExternalInput").ap()
    o_d = nc.dram_tensor("out", [BS, UNITS], FP32, kind="ExternalOutput").ap()

    with tile.TileContext(nc) as tc:
        with (
            tc.tile_pool(name="const", bufs=1) as cpool,
            tc.tile_pool(name="blk", bufs=2) as bpool,
            tc.tile_pool(name="psum", bufs=1, space="PSUM") as ppool,
        ):
            # DMAs in need order: u chunk A, silu weights, u chunk B,
            # then the two r-block weight chunks
            xu = cpool.tile([128, GK * 256], F16)
            nc.sync.dma_start(xu[:, 0:1024], xu_d[:, 0:1024])
            w2 = cpool.tile([128, KT, UNITS], F16)
            nc.sync.dma_start(w2[:, 0:2, :], w_d[:, 0:2, :])
            nc.sync.dma_start(xu[:, 1024:2048], xu_d[:, 1024:2048])
            nc.sync.dma_start(w2[:, 2:10, :], w_d[:, 2:10, :])
            nc.sync.dma_start(w2[:, 10:18, :], w_d[:, 10:18, :])

            # PE warm-up: HAM keeps the PE at 1.2 GHz until ~3.4us of
            # sustained activity; burn that in while the inputs stream
            wtile = cpool.tile([128, 512], F16)
            nc.vector.tensor_copy(
                wtile[:], nc.const_aps.tensor(1.0, (128, 512), FP32))
            wpsum = ppool.tile([128, 512], FP32)
            for _ in range(N_WARM):
                nc.tensor.matmul(wpsum[:], wtile[:, 0:128], wtile[:],
                                 start=True, stop=True)

            T = cpool.tile([128, NF * 256], F16)
            opsum = ppool.tile([BS, UNITS], FP32)

            # silu(x) = Silu(-u_0 - 0.6); k-tiles 0,1
            sbias = cpool.tile([128, 1], FP32)
            nc.vector.memset(sbias[:], -0.6)
            nc.scalar.activation(T[:, 0:256], xu[:, 0:256], Act.Silu,
                                 bias=sbias[:], scale=-1.0)
            nc.tensor.matmul(opsum[:], T[:, 0:128], w2[:, 0, :],
                             start=True, stop=False)
            nc.tensor.matmul(opsum[:], T[:, 128:256], w2[:, 1, :],
                             start=False, stop=False)

            # r blocks in two 4-block chunks: relu (DVE), square (ACT),
            # multiply (DVE), then the 8 matmuls of that chunk
            for h in range(2):
                u = xu[:, h * 1024:(h + 1) * 1024]
                t = bpool.tile([128, 1024], F16, tag="t")
                nc.vector.tensor_scalar_max(t[:], u, 0.0)
                s = bpool.tile([128, 1024], F16, tag="s")
                nc.scalar.square(s[:], u)
                blk = T[:, 256 + h * 1024:256 + (h + 1) * 1024]
                nc.vector.tensor_mul(blk, t[:], s[:])
                for i in range(8):
                    kt = 2 + 8 * h + i
                    nc.tensor.matmul(opsum[:],
                                     T[:, kt * 128:(kt + 1) * 128],
                                     w2[:, kt, :],
                                     start=False, stop=(kt == KT - 1))

            osb = cpool.tile([BS, UNITS], FP32)
            nc.vector.tensor_copy(osb[:], opsum[:])
            nc.sync.dma_start(o_d[:], osb[:])

    nc.compile()
    return nc


def _fold_weights(spline_kernel, scale_factor, bias):
    """-> (128, KT, UNITS) fp16 swizzled folded weights."""
    sk = spline_kernel.astype(np.float64)
    sf = scale_factor.astype(np.float64)
    b = bias.astype(np.float64)
    # W[i,j,o] = sk*sf + bias/IN  (bias folded via sum_j B_j == 1)
    W = sk * sf[:, None, :] + b[None, None, :] / IN
    comb = 2.5 ** 3 * np.array([1.0, -4.0, 6.0, -4.0, 1.0]) / 6.0
    # A[j, n] = coefficient of feature-block n in basis j
    A = np.zeros((GK, GK))
    for j in range(4):  # right-side: B_j = sum_m comb[m] * f_{j-m}
        for m in range(j + 1):
            A[j, j - m] = comb[m]
    for j in range(4, GK):  # left-side: B_j = sum_m comb[m] * f_{j+m}
        for m in range(GK - j):
            A[j, j + m] = comb[m]
    W2 = np.einsum("jn,ijo->nio", A, W)  # (GK, IN, UNITS)
    Wfull = np.concatenate([sf[None, :, :], W2], axis=0)  # silu block first
    flat = Wfull.reshape(K, UNITS)
    sw = flat.reshape(KT, 128, UNITS).transpose(1, 0, 2)  # -> [p, k, o]
    return np.ascontiguousarray(sw.astype(np.float16))


def _prep_x(x):
    """(BATCH, IN) -> per-core (128, GK*256) fp16 images [u_0 | .. | u_7]."""
    x = np.asarray(x, dtype=np.float32)
    outs = []
    for c in range(N_CORES):
        xs = x[c * BS:(c + 1) * BS]  # (BS, IN)
        xtc = np.ascontiguousarray(xs.T)  # (IN, BS)
        img = np.concatenate([xtc[:128], xtc[128:]], axis=1)  # (128, 256)
        blocks = []
        for n in range(GK):
            if n < 4:
                blocks.append((n - 1.5) / 2.5 - img)
            else:
                blocks.append(img + (5.5 - n) / 2.5)
        outs.append(np.ascontiguousarray(
            np.concatenate(blocks, axis=1).astype(np.float16)))
    return outs


def make_in_maps(inputs):
    w2 = _fold_weights(inputs["spline_kernel"], inputs["scale_factor"],
                       inputs["bias"])
    xus = _prep_x(inputs["x"])
    return [{"xu": xus[c], "w2": w2} for c in range(N_CORES)]


def kernel(x, spline_kernel, scale_factor, bias):
    if "nc" not in _cache:
        _cache["nc"] = _build()
    nc = _cache["nc"]

    in_maps = make_in_maps({"x": x, "spline_kernel": spline_kernel,
                            "scale_factor": scale_factor, "bias": bias})
    res = run_bass_kernel_spmd(nc, in_maps, list(range(N_CORES)))
    out = np.concatenate([res.results[c]["out"] for c in range(N_CORES)],
                         axis=0)
    return out.astype(np.float32)


# revision 6
# speedup vs baseline: 2.2383x; 1.0403x over previous
"""DenseKAN forward as a single fused fp16 matmul on TRN2.

Math: the reference uses a uniform knot grid (spacing h=0.4 on
[-2.2, 2.2]), so the Cox-de Boor bases are shifted copies of the
cardinal cubic B-spline; each basis B_j expands over truncated-power
features f_n = relu(u_n)^3 with

    u_n = (n-1.5)/2.5 - x   (n < 4,  right-side powers)
    u_n = x + (5.5-n)/2.5   (n >= 4, left-side powers)

plus a silu(x) block; all basis coefficients, the per-dim scale factor
and the bias (partition of unity, sum_j B_j == 1) fold into the weights
on the host, so the layer is out = [silu(x) | relu(u)^3 blocks] @ W2.

v2 pipeline (v1 bottleneck was 8 serialized GpSimd tensor_scalar ops at
~3.8us each, which also degraded concurrent DVE ops ~5x): the host
ships the 8 shifted blocks U = [u_0..u_7] as ONE [128, 2048] fp16 image
per core, so on-chip feature work is three WIDE ops per 4-block chunk —
DVE relu (tensor_scalar max), ACT square, DVE multiply — with no GpSimd
involvement. silu(x) is recovered from u_0 via ACT Silu(-u_0 - 0.6).
Everything (U, features, weights) is fp16: rel err ~2.6e-3 (vs 2e-2
budget), weight DMA halves vs fp32, and the PE runs at bf16 rate with
fast weight load. Batch is sharded across the 8 cores (128 rows each);
weights are replicated.
"""

import numpy as np

import concourse.bass as bass
import concourse.mybir as mybir
import concourse.tile as tile
from concourse import bacc
from concourse.bass_utils import run_bass_kernel_spmd

BATCH = 1024
IN = 256
UNITS = 256
GK = 8  # number of spline bases per input dim
NF = GK + 1  # + silu feature block
K = IN * NF  # 2304 contraction rows
N_CORES = 8
BS = BATCH // N_CORES  # 128 batch rows per core
KT = K // 128  # 18 k-tiles
N_WARM = 8  # PE warm-up matmuls (HAM clock-gate burn-in)

FP32 = mybir.dt.float32
F16 = mybir.dt.float16

AluOp = mybir.AluOpType
Act = mybir.ActivationFunctionType

_cache = {}


def _build():
    nc = bacc.Bacc("TRN2", target_bir_lowering=False, debug=False,
                   enable_asserts=False, num_devices=N_CORES)
    # host ships the 8 shifted blocks [u_0 | ... | u_7], each [128, 256]
    xu_d = nc.dram_tensor("xu", [128, GK * 256], F16,
                          kind="ExternalInput").ap()
    # host pre-swizzled: w2[p, k, o] = W2_flat[128*k + p, o], fp16
    w_d = nc.dram_tensor("w2", [128, KT, UNITS], F16,
                         kind="ExternalInput").ap()
    o_d = nc.dram_tensor("out", [BS, UNITS], FP32, kind="ExternalOutput").ap()

    with tile.TileContext(nc) as tc:
        with (
            tc.tile_pool(name="const", bufs=1) as cpool,
            tc.tile_pool(name="blk", bufs=2) as bpool,
            tc.tile_pool(name="psum", bufs=1, space="PSUM") as ppool,
        ):
            # DMAs on both HWDGE rings so issue costs don't serialize:
            # xu chunks on the SP ring, weights on the ACT ring.
            xu = cpool.tile([128, GK * 256], F16)
            w2 = cpool.tile([128, KT, UNITS], F16)
            nc.sync.dma_start(xu[:, 0:1024], xu_d[:, 0:1024])
            nc.scalar.dma_start(w2[:, 0:2, :], w_d[:, 0:2, :])
            nc.sync.dma_start(xu[:, 1024:2048], xu_d[:, 1024:2048])
            nc.scalar.dma_start(w2[:, 2:10, :], w_d[:, 2:10, :])
            nc.scalar.dma_start(w2[:, 10:18, :], w_d[:, 10:18, :])

            sbias = cpool.tile([128, 1], FP32)
            nc.vector.memset(sbias[:], -0.6)

            # force the SILU/SQUARE activation-table load off the
            # critical path: a dummy 1-col activation while DMAs stream
            dummy = cpool.tile([128, 1], F16)
            nc.scalar.activation(dummy[:], sbias[:], Act.Silu,
                                 bias=sbias[:], scale=-1.0)

            # PE warm-up: HAM keeps the PE at 1.2 GHz until ~3.4us of
            # sustained activity; burn that in while the inputs stream
            wtile = cpool.tile([128, 512], F16)
            nc.vector.tensor_copy(
                wtile[:], nc.const_aps.tensor(1.0, (128, 512), FP32))
            wpsum = ppool.tile([128, 512], FP32)
            for _ in range(N_WARM):
                nc.tensor.matmul(wpsum[:], wtile[:, 0:128], wtile[:],
                                 start=True, stop=True)

            T = cpool.tile([128, NF * 256], F16)
            opsum = ppool.tile([BS, UNITS], FP32)

            # silu(x) = Silu(-u_0 - 0.6); k-tiles 0,1
            nc.scalar.activation(T[:, 0:256], xu[:, 0:256], Act.Silu,
                                 bias=sbias[:], scale=-1.0)
            nc.tensor.matmul(opsum[:], T[:, 0:128], w2[:, 0, :],
                             start=True, stop=False)
            nc.tensor.matmul(opsum[:], T[:, 128:256], w2[:, 1, :],
                             start=False, stop=False)

            # r blocks in two 4-block chunks: relu (DVE), square (ACT
            # for chunk 0, DVE for chunk 1 — balances the engines),
            # multiply (DVE), then the 8 matmuls of that chunk
            for h in range(2):
                u = xu[:, h * 1024:(h + 1) * 1024]
                t = bpool.tile([128, 1024], F16, tag="t")
                nc.vector.tensor_scalar_max(t[:], u, 0.0)
                s = bpool.tile([128, 1024], F16, tag="s")
                if h == 0:
                    nc.scalar.square(s[:], u)
                else:
                    nc.vector.tensor_mul(s[:], u, u)
                blk = T[:, 256 + h * 1024:256 + (h + 1) * 1024]
                nc.vector.tensor_mul(blk, t[:], s[:])
                for i in range(8):
                    kt = 2 + 8 * h + i
                    nc.tensor.matmul(opsum[:],
                                     T[:, kt * 128:(kt + 1) * 128],
                                     w2[:, kt, :],
                                     start=False, stop=(kt == KT - 1))

            # split the output copy/DMA so the first half's store
            # overlaps the second half's PSUM read
            osb = cpool.tile([BS, UNITS], FP32)
            nc.vector.tensor_copy(osb[:, 0:128], opsum[:, 0:128])
            nc.sync.dma_start(o_d[:, 0:128], osb[:, 0:128])
            nc.vector.tensor_copy(osb[:, 128:256], opsum[:, 128:256])
            nc.sync.dma_start(o_d[:, 128:256], osb[:, 128:256])

    nc.compile()
    return nc


def _fold_weights(spline_kernel, scale_factor, bias):
    """-> (128, KT, UNITS) fp16 swizzled folded weights."""
    sk = spline_kernel.astype(np.float64)
    sf = scale_factor.astype(np.float64)
    b = bias.astype(np.float64)
    # W[i,j,o] = sk*sf + bias/IN  (bias folded via sum_j B_j == 1)
    W = sk * sf[:, None, :] + b[None, None, :] / IN
    comb = 2.5 ** 3 * np.array([1.0, -4.0, 6.0, -4.0, 1.0]) / 6.0
    # A[j, n] = coefficient of feature-block n in basis j
    A = np.zeros((GK, GK))
    for j in range(4):  # right-side: B_j = sum_m comb[m] * f_{j-m}
        for m in range(j + 1):
            A[j, j - m] = comb[m]
    for j in range(4, GK):  # left-side: B_j = sum_m comb[m] * f_{j+m}
        for m in range(GK - j):
            A[j, j + m] = comb[m]
    W2 = np.einsum("jn,ijo->nio", A, W)  # (GK, IN, UNITS)
    Wfull = np.concatenate([sf[None, :, :], W2], axis=0)  # silu block first
    flat = Wfull.reshape(K, UNITS)
    sw = flat.reshape(KT, 128, UNITS).transpose(1, 0, 2)  # -> [p, k, o]
    return np.ascontiguousarray(sw.astype(np.float16))


def _prep_x(x):
    """(BATCH, IN) -> per-core (128, GK*256) fp16 images [u_0 | .. | u_7]."""
    x = np.asarray(x, dtype=np.float32)
    outs = []
    for c in range(N_CORES):
        xs = x[c * BS:(c + 1) * BS]  # (BS, IN)
        xtc = np.ascontiguousarray(xs.T)  # (IN, BS)
        img = np.concatenate([xtc[:128], xtc[128:]], axis=1)  # (128, 256)
        blocks = []
        for n in range(GK):
            if n < 4:
                blocks.append((n - 1.5) / 2.5 - img)
            else:
                blocks.append(img + (5.5 - n) / 2.5)
        outs.append(np.ascontiguousarray(
            np.concatenate(blocks, axis=1).astype(np.float16)))
    return outs


def make_in_maps(inputs):
    w2 = _fold_weights(inputs["spline_kernel"], inputs["scale_factor"],
                       inputs["bias"])
    xus = _prep_x(inputs["x"])
    return [{"xu": xus[c], "w2": w2} for c in range(N_CORES)]


def kernel(x, spline_kernel, scale_factor, bias):
    if "nc" not in _cache:
        _cache["nc"] = _build()
    nc = _cache["nc"]

    in_maps = make_in_maps({"x": x, "spline_kernel": spline_kernel,
                            "scale_factor": scale_factor, "bias": bias})
    res = run_bass_kernel_spmd(nc, in_maps, list(range(N_CORES)))
    out = np.concatenate([res.results[c]["out"] for c in range(N_CORES)],
                         axis=0)
    return out.astype(np.float32)
